# revision 45
# baseline (speedup 1.0000x reference)
"""Trainium2 Bass kernel v4 for nn_CudaMixedBitLinear (GPTQ-style 4-bit linear).

out[b,s,o] = sum_k x[b,s,k] * W[o,k],  W[o,k] = (q[o,k] - z[o,g]) * s[o,g],
g = k // 128, q/z packed as nibbles (low nibble first) in int32 bytes.

Sharding: column-parallel over out_features across 8 cores (11008 -> 1376
per core), x replicated, outputs concatenated on host. No collectives.

v4 (from v3 at 589us, v2 fp16 at 639us): fp8 DoubleRow matmuls with the
phase-A pipeline rebalanced so the PE (the roofline engine at 460us busy)
never starves:
  - PE path: fp8e4m3 DoubleRow, 0.5 cycles/row, 256-k contraction per
    instruction.  Precision: 3 products per k-pair —
    (x_hi + x_lo) @ w_hi + x_hi @ w_lo — all sharing one PSUM
    accumulation at a global w-scale of 64 (rel err 1.3e-3 measured).
  - x is pre-split hi/lo fp8 on host, shipped block-contiguous
    [128, nb, jt, i, m] so each x-block load is 128 descriptors, not 4096
    (SWDGE descriptor generation runs on the Pool engine).
  - weight dequant per o-tile: DVE does unpack + 32 group scales +
    the w_lo subtract (fp16-fp8 mixed, 1x); the w_hi fp16->fp8 convert
    rides ACT (activation Copy); hi-transpose on ACT HWDGE, lo-transpose
    on SP HWDGE.  Transposes move uint16 PAIRS of adjacent-k fp8 values
    (1-byte dtypes can't XBAR-transpose; the pair layout is exactly the
    DoubleRow operand layout).
  - phase-A drains stage fp16 pieces into an SBUF tile per o-tile and
    leave as ONE Pool DMA per o-tile (11 DMAs instead of 88).
  - output tensor is fp16 (host upconverts to f32; adds ~3e-4 in
    quadrature), halving store traffic.
"""

import numpy as np

B, S, K = 2, 2048, 4096
OUT_F = 11008
N_CORES = 8
OC = OUT_F // N_CORES       # 1376 out features per core
GROUP = 128
GROUPS = K // GROUP         # 32
M = B * S                   # 4096 rows
JT = K // 256               # 16 pair-tiles (256 k's each)
OT = (OC + 127) // 128      # 11 o-tiles (last has 96 rows)
XB = 256                    # m columns per x block buffer
NB = M // XB                # 16 m-blocks
SUBS = XB // 128            # 2 m-tiles per block
WSCALE = 64.0               # w pre-scale so e4m3 stays in normal range
AUNITS = 10                 # phase A covers m-tiles 0..9 (m-blocks 0-4)
XRING = 5                   # x block ring depth

_CACHE = {}
RUN_KWARGS = {}   # test harness can inject e.g. dict(trace=True)
LAST_RESULT = None


def _build_bass():
    import concourse.bass as bass
    import concourse.bacc as bacc
    import concourse.mybir as mybir
    from concourse.tile import TileContext

    A = mybir.AluOpType
    fp16 = mybir.dt.float16
    f32 = mybir.dt.float32
    i32 = mybir.dt.int32
    i16 = mybir.dt.int16
    fp8 = mybir.dt.float8e4
    u16 = mybir.dt.uint16
    DR = mybir.MatmulPerfMode.DoubleRow

    nc = bacc.Bacc("TRN2", target_bir_lowering=False)
    xhi_d = nc.dram_tensor("xhi", [128, NB * JT * 2 * XB], fp8,
                           kind="ExternalInput")
    xlo_d = nc.dram_tensor("xlo", [128, NB * JT * 2 * XB], fp8,
                           kind="ExternalInput")
    qw = nc.dram_tensor("qweight", [OC, K // 2], i32, kind="ExternalInput")
    sc = nc.dram_tensor("scales", [OC, GROUPS], fp16, kind="ExternalInput")
    qz = nc.dram_tensor("qzeros", [OC, GROUPS // 2], i32, kind="ExternalInput")
    out = nc.dram_tensor("out", [M, OC], fp16, kind="ExternalOutput")

    # static SBUF: resident w^T hi/lo (uint16 pairs), x^T rings, out ring
    wthi = nc.alloc_sbuf_tensor("wthi", [128, JT * OC], u16).ap()
    wtlo = nc.alloc_sbuf_tensor("wtlo", [128, JT * OC], u16).ap()
    wthi3 = wthi.rearrange("p (jt oc) -> p jt oc", jt=JT)
    wtlo3 = wtlo.rearrange("p (jt oc) -> p jt oc", jt=JT)
    # fp8 views for the matmul rhs: [p, jt, i, o], i stride 1, o stride 2
    wthi_v = wthi.bitcast(fp8).rearrange("p (jt oc i) -> p jt i oc", jt=JT, i=2)
    wtlo_v = wtlo.bitcast(fp8).rearrange("p (jt oc i) -> p jt i oc", jt=JT, i=2)

    xhs = [nc.alloc_sbuf_tensor(f"xhbuf{i}", [128, JT, 2, XB], fp8).ap()
           for i in range(XRING)]
    xls = [nc.alloc_sbuf_tensor(f"xlbuf{i}", [128, JT, 2, XB], fp8).ap()
           for i in range(XRING)]
    obs = [nc.alloc_sbuf_tensor(f"obbuf{i}", [128, OC], fp16).ap()
           for i in range(2)]

    # x DRAM views: block-contiguous [p, nb, jt, i, m]
    xhi_v5 = xhi_d[:, :].rearrange("p (nb jt i m) -> p nb jt i m",
                                   nb=NB, jt=JT, i=2)
    xlo_v5 = xlo_d[:, :].rearrange("p (nb jt i m) -> p nb jt i m",
                                   nb=NB, jt=JT, i=2)
    with TileContext(nc) as tc:
        with (
            tc.tile_pool(name="deq", bufs=3) as deq,
            tc.tile_pool(name="qp", bufs=2) as qp,
            tc.tile_pool(name="w8p", bufs=2) as w8p,
            tc.tile_pool(name="wlo", bufs=2) as wlop,
            tc.tile_pool(name="stg", bufs=8) as stgp,
            tc.tile_pool(name="pCol", bufs=4, space="PSUM") as pcol,
            tc.tile_pool(name="psB", bufs=2, space="PSUM") as ppB,
        ):
            # ------------- phase A: load one o-tile ---------------------------
            loaded = {}

            def emit_loads(t, nslices=1):
                eng = nc.sync
                o0 = t * 128
                osz = min(128, OC - o0)
                s_t = deq.tile([128, GROUPS], fp16, tag="s", name="s_t")
                eng.dma_start(out=s_t[:osz], in_=sc[o0:o0 + osz, :])
                z_t = deq.tile([128, GROUPS // 2], i32, tag="z", name="z_t")
                eng.dma_start(out=z_t[:osz], in_=qz[o0:o0 + osz, :])
                q_t = qp.tile([128, K // 2], i32, tag="q", name="q_t")
                ksl = (K // 2) // nslices
                for sl in range(nslices):
                    eng.dma_start(out=q_t[:osz, sl * ksl:(sl + 1) * ksl],
                                  in_=qw[o0:o0 + osz, sl * ksl:(sl + 1) * ksl])
                loaded[t] = (s_t, z_t, q_t)

            # ------------- phase A: dequant one o-tile (k-sliced) -------------
            def emit_deq(t, nslices):
                o0 = t * 128
                osz = min(128, OC - o0)
                s_t, z_t, q_t = loaded.pop(t)

                # zeros -> fp16(1024+z) via mantissa-OR trick (i32 lanes)
                z1 = deq.tile([128, GROUPS // 2], i32, tag="z1", name="z1")
                z2 = deq.tile([128, GROUPS // 2], i32, tag="z2", name="z2")
                nc.vector.tensor_scalar(out=z1[:osz], in0=z_t[:osz], scalar1=15,
                                        scalar2=0x64006400, op0=A.bitwise_and,
                                        op1=A.bitwise_or)
                nc.vector.tensor_scalar(out=z2[:osz], in0=z_t[:osz], scalar1=12,
                                        scalar2=0x000F0000,
                                        op0=A.logical_shift_left, op1=A.bitwise_and)
                nc.vector.tensor_tensor(out=z1[:osz], in0=z1[:osz], in1=z2[:osz],
                                        op=A.bitwise_or)
                zf = z1.bitcast(fp16)   # [128, GROUPS] = 1024 + z

                # per-group fp32 scalars: s64 = 64*s ; zs64 = -(1024+z)*64*s
                s64 = deq.tile([128, GROUPS], f32, tag="s64", name="s64")
                nc.vector.tensor_scalar(out=s64[:osz], in0=s_t[:osz],
                                        scalar1=WSCALE, scalar2=None, op0=A.mult)
                zs64 = deq.tile([128, GROUPS], f32, tag="zs64", name="zs64")
                nc.vector.tensor_tensor(out=zs64[:osz], in0=zf[:osz],
                                        in1=s_t[:osz], op=A.mult)
                nc.vector.tensor_scalar(out=zs64[:osz], in0=zs64[:osz],
                                        scalar1=-WSCALE, scalar2=None, op0=A.mult)

                # junky unpack fully in place over the q tile; then per-group
                # dequant in place (fp16, scaled by 64).
                q16 = q_t.bitcast(i16)          # [128, K] lanes (b, 0, ...)
                q16_2 = q16.rearrange("p (c two) -> p c two", two=2)
                vf16 = q_t.bitcast(fp16)        # [128, K]
                whi8 = w8p.tile([128, K], fp8, tag="whi", name="whi8")
                wlo8 = wlop.tile([128, K], fp8, tag="wlo", name="wlo8")
                whi_u16 = whi8.bitcast(u16)     # [128, K//2]
                wlo_u16 = wlo8.bitcast(u16)
                gsl = GROUPS // nslices
                for sl in range(nslices):
                    ca, cb = sl * (K // nslices), (sl + 1) * (K // nslices)
                    # op2: odd lanes = (even_lane >> 4) | 0x6400
                    nc.vector.tensor_scalar(
                        out=q16_2[:osz, ca // 2:cb // 2, 1],
                        in0=q16_2[:osz, ca // 2:cb // 2, 0],
                        scalar1=4, scalar2=0x6400,
                        op0=A.logical_shift_right, op1=A.bitwise_or)
                    # op1: all i16 lanes: (v & 15) | 0x6400
                    nc.vector.tensor_scalar(out=q16[:osz, ca:cb],
                                            in0=q16[:osz, ca:cb],
                                            scalar1=15, scalar2=0x6400,
                                            op0=A.bitwise_and, op1=A.bitwise_or)
                    # dequant in place: w64 = vf * s64 + zs64  (== 64(q-z)s)
                    for g in range(sl * gsl, (sl + 1) * gsl):
                        nc.vector.tensor_scalar(
                            out=vf16[:osz, g * GROUP:(g + 1) * GROUP],
                            in0=vf16[:osz, g * GROUP:(g + 1) * GROUP],
                            scalar1=s64[:osz, g:g + 1],
                            scalar2=zs64[:osz, g:g + 1],
                            op0=A.mult, op1=A.add)
                for sl in range(nslices):
                    ca, cb = sl * (K // nslices), (sl + 1) * (K // nslices)
                    jta, jtb = sl * (JT // nslices), (sl + 1) * (JT // nslices)
                    # fp8 hi on ACT, then its transpose on ACT HWDGE
                    nc.scalar.copy(out=whi8[:osz, ca:cb], in_=vf16[:osz, ca:cb])
                    nc.scalar.dma_start(out=wthi3[:, jta:jtb, o0:o0 + osz],
                                        in_=whi_u16[:osz, ca // 2:cb // 2],
                                        transpose=True)
                    # residual lo on DVE (mixed fp16-fp8 subtract, 1x), then
                    # its transpose on SP HWDGE
                    nc.vector.tensor_tensor(out=wlo8[:osz, ca:cb],
                                            in0=vf16[:osz, ca:cb],
                                            in1=whi8[:osz, ca:cb],
                                            op=A.subtract)
                    nc.sync.dma_start(out=wtlo3[:, jta:jtb, o0:o0 + osz],
                                      in_=wlo_u16[:osz, ca // 2:cb // 2],
                                      transpose=True)

            # ------- phase A: DR matmuls for up to 4 (ot, mtile) units --------
            pending_drains = []

            def emit_unit_mms(units):
                # one PSUM bank per unit (start=True zeroes the whole bank's
                # zero region, so independent accumulations never share)
                tiles = [pcol.tile([128, 512], f32, tag="pc", name="pc")
                         for _ in units]
                for st in range(3):
                    for jt in range(JT):
                        for (t, mi), ps in zip(units, tiles):
                            o0 = t * 128
                            osz = min(128, OC - o0)
                            bi = (mi // SUBS) % XRING
                            sub = mi % SUBS
                            xv = xhs[bi] if st != 1 else xls[bi]
                            wv = wthi_v if st != 2 else wtlo_v
                            nc.tensor.matmul(
                                ps[:, :osz],
                                lhsT=xv[:, jt, :, sub * 128:(sub + 1) * 128],
                                rhs=wv[:, jt, :, o0:o0 + osz],
                                start=(st == 0 and jt == 0),
                                stop=(st == 2 and jt == JT - 1),
                                perf_mode=DR)
                pending_drains.append((tiles, list(units)))

            def flush_drains(keep=0):
                while len(pending_drains) > keep:
                    tiles, units = pending_drains.pop(0)
                    for (t, mi), ps in zip(units, tiles):
                        o0 = t * 128
                        osz = min(128, OC - o0)
                        st = stgp.tile([128, 128], fp16, tag="st", name="st")
                        nc.scalar.mul(out=st[:, :osz], in_=ps[:, :osz],
                                      mul=1.0 / WSCALE)
                        nc.gpsimd.dma_start(
                            out=out[mi * 128:(mi + 1) * 128, o0:o0 + osz],
                            in_=st[:, :osz])

            # ------------- phase B: one full m-block -------------------------
            def emit_mblock(mb, load_xt=True):
                xh, xl = xhs[mb % XRING], xls[mb % XRING]
                if load_xt:
                    emit_xt(mb, 1)
                for sub in range(SUBS):
                    mi = mb * SUBS + sub
                    last = (mb == NB - 1 and sub == SUBS - 1)
                    chunks = [(0, 512), (512, 1024), (1024, OC)]
                    psts = [ppB.tile([128, 512], f32, tag=f"pp{j}",
                                     name=f"pp{j}") for j in range(2)]
                    psts.append(pcol.tile([128, 512], f32, tag="pc",
                                          name="pc"))
                    ob = obs[mi % 2]
                    if last:
                        # j-outer: early chunks drain while later ones matmul
                        for j, (c0, c1) in enumerate(chunks):
                            for st in range(3):
                                xv = xh if st != 1 else xl
                                wv = wthi_v if st != 2 else wtlo_v
                                for jt in range(JT):
                                    nc.tensor.matmul(
                                        psts[j][:, :c1 - c0],
                                        lhsT=xv[:, jt, :,
                                                sub * 128:(sub + 1) * 128],
                                        rhs=wv[:, jt, :, c0:c1],
                                        start=(st == 0 and jt == 0),
                                        stop=(st == 2 and jt == JT - 1),
                                        perf_mode=DR)
                            nc.scalar.mul(out=ob[:, c0:c1],
                                          in_=psts[j][:, :c1 - c0],
                                          mul=1.0 / WSCALE)
                            nc.scalar.dma_start(
                                out=out[mi * 128:(mi + 1) * 128, c0:c1],
                                in_=ob[:, c0:c1])
                    else:
                        for st in range(3):
                            xv = xh if st != 1 else xl
                            wv = wthi_v if st != 2 else wtlo_v
                            for jt in range(JT):
                                for j, (c0, c1) in enumerate(chunks):
                                    nc.tensor.matmul(
                                        psts[j][:, :c1 - c0],
                                        lhsT=xv[:, jt, :,
                                                sub * 128:(sub + 1) * 128],
                                        rhs=wv[:, jt, :, c0:c1],
                                        start=(st == 0 and jt == 0),
                                        stop=(st == 2 and jt == JT - 1),
                                        perf_mode=DR)
                        for j, (c0, c1) in enumerate(chunks):
                            nc.scalar.mul(out=ob[:, c0:c1],
                                          in_=psts[j][:, :c1 - c0],
                                          mul=1.0 / WSCALE)
                        nc.gpsimd.dma_start(out=out[mi * 128:(mi + 1) * 128, :],
                                            in_=ob)

            # ---------------- emission schedule ----------------
            def emit_xt(block, parts):
                xh, xl = xhs[block % XRING], xls[block % XRING]
                jp = JT // parts
                for part in range(parts):
                    ja, jb = part * jp, (part + 1) * jp
                    nc.gpsimd.dma_start(out=xh[:, ja:jb, :, :],
                                        in_=xhi_v5[:, block, ja:jb, :, :])
                    nc.gpsimd.dma_start(out=xl[:, ja:jb, :, :],
                                        in_=xlo_v5[:, block, ja:jb, :, :])

            # warm the ACT function table at t=0 so the auto-inserted
            # LoadActFuncSet doesn't sit in front of the first whi8 converts
            warm = deq.tile([128, 2], fp16, tag="warm", name="warm")
            nc.vector.memset(warm[:1, :2], 0.0)
            nc.scalar.copy(out=warm[:1, 1:2], in_=warm[:1, :1])

            # phase A: o-tiles with interleaved DR matmuls over m-blocks 0-4.
            SLICES = {0: 4, 1: 4, 2: 4, 3: 2}
            emit_loads(0, 4)
            emit_loads(1, 4)
            emit_xt(0, 4)
            emit_xt(1, 4)
            emit_deq(0, SLICES[0])
            emit_loads(2, 4)
            emit_deq(1, SLICES[1])
            emit_unit_mms([(0, 0), (0, 1)])
            emit_unit_mms([(0, 2), (0, 3)])
            flush_drains(keep=1)
            emit_loads(3, 2)
            emit_xt(2, 2)
            emit_deq(2, SLICES[2])
            emit_unit_mms([(1, 0), (1, 1)])
            flush_drains(keep=1)
            emit_unit_mms([(1, 2), (1, 3)])
            flush_drains(keep=1)
            emit_loads(4, 1)
            emit_xt(3, 2)
            emit_deq(3, SLICES[3])
            emit_unit_mms([(0, 4), (0, 5)])
            flush_drains(keep=1)
            emit_unit_mms([(1, 4), (1, 5)])
            flush_drains(keep=1)
            emit_unit_mms([(2, 0), (2, 1)])
            flush_drains(keep=1)
            emit_unit_mms([(2, 2), (2, 3)])
            flush_drains(keep=1)
            emit_xt(4, 2)
            emit_unit_mms([(0, 6), (0, 7)])
            flush_drains(keep=1)
            emit_unit_mms([(1, 6), (1, 7)])
            flush_drains(keep=1)
            emit_unit_mms([(2, 4), (2, 5)])
            flush_drains(keep=1)
            emit_unit_mms([(0, 8), (0, 9)])
            flush_drains(keep=1)
            emit_unit_mms([(1, 8), (1, 9)])
            flush_drains(keep=1)
            emit_unit_mms([(2, 6), (2, 7)])
            flush_drains(keep=1)
            emit_unit_mms([(2, 8), (2, 9)])
            flush_drains(keep=1)
            for t in range(4, OT):
                if t + 1 < OT:
                    emit_loads(t + 1, 1)
                emit_deq(t, 2)
                for mi0 in range(0, AUNITS, 2):
                    emit_unit_mms([(t - 1, mi0), (t - 1, mi0 + 1)])
                    flush_drains(keep=1)
            for mi0 in range(0, AUNITS, 2):
                emit_unit_mms([(OT - 1, mi0), (OT - 1, mi0 + 1)])
                flush_drains(keep=1)
            flush_drains(keep=0)

            # phase B: m-blocks 5..15
            for mb in range(XRING, NB):
                emit_mblock(mb)

    if not nc.is_finalized():
        nc.finalize()
    return nc


def kernel(x, qweight, scales, qzeros, group_size=128, **_unused):
    global LAST_RESULT
    import ml_dtypes
    from concourse.bass_utils import run_bass_kernel_spmd

    e4 = ml_dtypes.float8_e4m3

    if "nc" not in _CACHE:
        _CACHE["nc"] = _build_bass()
    nc = _CACHE["nc"]

    x2d = np.asarray(x).reshape(M, K)
    xT = np.ascontiguousarray(x2d.T).astype(np.float32)   # [K, M]
    x_hi = xT.astype(e4)
    x_lo = (xT - x_hi.astype(np.float32)).astype(e4)

    def pack(a):
        # [K, M] fp8 -> [128, nb, jt, i, m] block-contiguous,
        # k = 256*jt + 2*p + i, m = nb*XB + m'
        b = a.reshape(JT, 128, 2, NB, XB)          # jt, p, i, nb, xb
        return np.ascontiguousarray(
            b.transpose(1, 3, 0, 2, 4).reshape(128, -1)).view(np.uint8)

    xhi_p = pack(x_hi)
    xlo_p = pack(x_lo)
    qweight = np.asarray(qweight)
    scales = np.asarray(scales)
    qzeros = np.asarray(qzeros)

    in_maps = []
    for i in range(N_CORES):
        sl = slice(i * OC, (i + 1) * OC)
        in_maps.append({
            "xhi": xhi_p,
            "xlo": xlo_p,
            "qweight": np.ascontiguousarray(qweight[sl]),
            "scales": np.ascontiguousarray(scales[sl]),
            "qzeros": np.ascontiguousarray(qzeros[sl]),
        })

    res = run_bass_kernel_spmd(nc, in_maps, core_ids=list(range(N_CORES)),
                               **RUN_KWARGS)
    LAST_RESULT = res
    outs = [r["out"] for r in res.results]
    return np.concatenate(outs, axis=1).reshape(B, S, OUT_F).astype(np.float32)


# revision 54
# speedup vs baseline: 1.0793x; 1.0793x over previous
"""Trainium2 Bass kernel v4 for nn_CudaMixedBitLinear (GPTQ-style 4-bit linear).

out[b,s,o] = sum_k x[b,s,k] * W[o,k],  W[o,k] = (q[o,k] - z[o,g]) * s[o,g],
g = k // 128, q/z packed as nibbles (low nibble first) in int32 bytes.

Sharding: column-parallel over out_features across 8 cores (11008 -> 1376
per core), x replicated, outputs concatenated on host. No collectives.

v4 (from v3 at 589us, v2 fp16 at 639us): fp8 DoubleRow matmuls with the
phase-A pipeline rebalanced so the PE (the roofline engine at 460us busy)
never starves:
  - PE path: fp8e4m3 DoubleRow, 0.5 cycles/row, 256-k contraction per
    instruction.  Precision: 3 products per k-pair —
    (x_hi + x_lo) @ w_hi + x_hi @ w_lo — all sharing one PSUM
    accumulation at a global w-scale of 64 (rel err 1.3e-3 measured).
  - x is pre-split hi/lo fp8 on host, shipped block-contiguous
    [128, nb, jt, i, m] so each x-block load is 128 descriptors, not 4096
    (SWDGE descriptor generation runs on the Pool engine).
  - weight dequant per o-tile: DVE does unpack + 32 group scales +
    the w_lo subtract (fp16-fp8 mixed, 1x); the w_hi fp16->fp8 convert
    rides ACT (activation Copy); hi-transpose on ACT HWDGE, lo-transpose
    on SP HWDGE.  Transposes move uint16 PAIRS of adjacent-k fp8 values
    (1-byte dtypes can't XBAR-transpose; the pair layout is exactly the
    DoubleRow operand layout).
  - phase-A drains stage fp16 pieces into an SBUF tile per o-tile and
    leave as ONE Pool DMA per o-tile (11 DMAs instead of 88).
  - output tensor is fp16 (host upconverts to f32; adds ~3e-4 in
    quadrature), halving store traffic.
"""

import numpy as np

B, S, K = 2, 2048, 4096
OUT_F = 11008
N_CORES = 8
OC = OUT_F // N_CORES       # 1376 out features per core
GROUP = 128
GROUPS = K // GROUP         # 32
M = B * S                   # 4096 rows
JT = K // 256               # 16 pair-tiles (256 k's each)
OT = (OC + 127) // 128      # 11 o-tiles (last has 96 rows)
XB = 256                    # m columns per x block buffer
NB = M // XB                # 16 m-blocks
SUBS = XB // 128            # 2 m-tiles per block
WSCALE = 64.0               # w pre-scale so e4m3 stays in normal range
AUNITS = 8                  # phase A covers m-tiles 0..7 (m-blocks 0-3)
XRING = 4                   # x block ring depth

_CACHE = {}
RUN_KWARGS = {}   # test harness can inject e.g. dict(trace=True)
LAST_RESULT = None


def _build_bass():
    import concourse.bass as bass
    import concourse.bacc as bacc
    import concourse.mybir as mybir
    from concourse.tile import TileContext

    A = mybir.AluOpType
    fp16 = mybir.dt.float16
    f32 = mybir.dt.float32
    i32 = mybir.dt.int32
    i16 = mybir.dt.int16
    fp8 = mybir.dt.float8e4
    u16 = mybir.dt.uint16
    DR = mybir.MatmulPerfMode.DoubleRow

    nc = bacc.Bacc("TRN2", target_bir_lowering=False)
    xhi_d = nc.dram_tensor("xhi", [128, NB * JT * 2 * XB], fp8,
                           kind="ExternalInput")
    xlo_d = nc.dram_tensor("xlo", [128, NB * JT * 2 * XB], fp8,
                           kind="ExternalInput")
    qw = nc.dram_tensor("qweight", [OC, K // 2], i32, kind="ExternalInput")
    sc = nc.dram_tensor("scales", [OC, GROUPS], fp16, kind="ExternalInput")
    qz = nc.dram_tensor("qzeros", [OC, GROUPS // 2], i32, kind="ExternalInput")
    out = nc.dram_tensor("out", [M, OC], fp16, kind="ExternalOutput")

    # static SBUF: resident w^T hi/lo (uint16 pairs), x^T rings, out ring
    wthi = nc.alloc_sbuf_tensor("wthi", [128, JT * OC], u16).ap()
    wtlo = nc.alloc_sbuf_tensor("wtlo", [128, JT * OC], u16).ap()
    wthi3 = wthi.rearrange("p (jt oc) -> p jt oc", jt=JT)
    wtlo3 = wtlo.rearrange("p (jt oc) -> p jt oc", jt=JT)
    # fp8 views for the matmul rhs: [p, jt, i, o], i stride 1, o stride 2
    wthi_v = wthi.bitcast(fp8).rearrange("p (jt oc i) -> p jt i oc", jt=JT, i=2)
    wtlo_v = wtlo.bitcast(fp8).rearrange("p (jt oc i) -> p jt i oc", jt=JT, i=2)

    xhs = [nc.alloc_sbuf_tensor(f"xhbuf{i}", [128, JT, 2, XB], fp8).ap()
           for i in range(XRING)]
    xls = [nc.alloc_sbuf_tensor(f"xlbuf{i}", [128, JT, 2, XB], fp8).ap()
           for i in range(XRING)]
    obs = [nc.alloc_sbuf_tensor(f"obbuf{i}", [128, OC], fp16).ap()
           for i in range(2)]

    # x DRAM views: block-contiguous [p, nb, jt, i, m]
    xhi_v5 = xhi_d[:, :].rearrange("p (nb jt i m) -> p nb jt i m",
                                   nb=NB, jt=JT, i=2)
    xlo_v5 = xlo_d[:, :].rearrange("p (nb jt i m) -> p nb jt i m",
                                   nb=NB, jt=JT, i=2)
    # phase-A staged-output DRAM view: rows 0..AUNITS*128 as [p, mi, o]
    outA_v = out[0:AUNITS * 128, :].rearrange("(mi p) o -> p mi o", p=128)
    with TileContext(nc) as tc:
        with (
            tc.tile_pool(name="deq", bufs=3) as deq,
            tc.tile_pool(name="qp", bufs=3) as qp,
            tc.tile_pool(name="w8p", bufs=2) as w8p,
            tc.tile_pool(name="wlo", bufs=2) as wlop,
            tc.tile_pool(name="stgA", bufs=4) as stgp,
            tc.tile_pool(name="pCol", bufs=4, space="PSUM") as pcol,
            tc.tile_pool(name="psB", bufs=2, space="PSUM") as ppB,
        ):
            # ------------- phase A: load one o-tile ---------------------------
            loaded = {}

            def emit_loads(t, nslices=1):
                eng = nc.sync
                o0 = t * 128
                osz = min(128, OC - o0)
                s_t = deq.tile([128, GROUPS], fp16, tag="s", name="s_t")
                eng.dma_start(out=s_t[:osz], in_=sc[o0:o0 + osz, :])
                z_t = deq.tile([128, GROUPS // 2], i32, tag="z", name="z_t")
                eng.dma_start(out=z_t[:osz], in_=qz[o0:o0 + osz, :])
                q_t = qp.tile([128, K // 2], i32, tag="q", name="q_t")
                ksl = (K // 2) // nslices
                for sl in range(nslices):
                    eng.dma_start(out=q_t[:osz, sl * ksl:(sl + 1) * ksl],
                                  in_=qw[o0:o0 + osz, sl * ksl:(sl + 1) * ksl])
                loaded[t] = (s_t, z_t, q_t)

            # ------------- phase A: dequant one o-tile (k-sliced) -------------
            def emit_deq(t, nslices):
                o0 = t * 128
                osz = min(128, OC - o0)
                s_t, z_t, q_t = loaded.pop(t)

                # zeros -> fp16(1024+z) via mantissa-OR trick (i32 lanes)
                z1 = deq.tile([128, GROUPS // 2], i32, tag="z1", name="z1")
                z2 = deq.tile([128, GROUPS // 2], i32, tag="z2", name="z2")
                nc.vector.tensor_scalar(out=z1[:osz], in0=z_t[:osz], scalar1=15,
                                        scalar2=0x64006400, op0=A.bitwise_and,
                                        op1=A.bitwise_or)
                nc.vector.tensor_scalar(out=z2[:osz], in0=z_t[:osz], scalar1=12,
                                        scalar2=0x000F0000,
                                        op0=A.logical_shift_left, op1=A.bitwise_and)
                nc.vector.tensor_tensor(out=z1[:osz], in0=z1[:osz], in1=z2[:osz],
                                        op=A.bitwise_or)
                zf = z1.bitcast(fp16)   # [128, GROUPS] = 1024 + z

                # per-group fp32 scalars: s64 = 64*s ; zs64 = -(1024+z)*64*s
                s64 = deq.tile([128, GROUPS], f32, tag="s64", name="s64")
                nc.vector.tensor_scalar(out=s64[:osz], in0=s_t[:osz],
                                        scalar1=WSCALE, scalar2=None, op0=A.mult)
                zs64 = deq.tile([128, GROUPS], f32, tag="zs64", name="zs64")
                nc.vector.tensor_tensor(out=zs64[:osz], in0=zf[:osz],
                                        in1=s_t[:osz], op=A.mult)
                nc.vector.tensor_scalar(out=zs64[:osz], in0=zs64[:osz],
                                        scalar1=-WSCALE, scalar2=None, op0=A.mult)

                # junky unpack fully in place over the q tile; then per-group
                # dequant in place (fp16, scaled by 64).
                q16 = q_t.bitcast(i16)          # [128, K] lanes (b, 0, ...)
                q16_2 = q16.rearrange("p (c two) -> p c two", two=2)
                vf16 = q_t.bitcast(fp16)        # [128, K]
                whi8 = w8p.tile([128, K], fp8, tag="whi", name="whi8")
                wlo8 = wlop.tile([128, K], fp8, tag="wlo", name="wlo8")
                whi_u16 = whi8.bitcast(u16)     # [128, K//2]
                wlo_u16 = wlo8.bitcast(u16)
                gsl = GROUPS // nslices
                for sl in range(nslices):
                    ca, cb = sl * (K // nslices), (sl + 1) * (K // nslices)
                    # op2: odd lanes = (even_lane >> 4) | 0x6400
                    nc.vector.tensor_scalar(
                        out=q16_2[:osz, ca // 2:cb // 2, 1],
                        in0=q16_2[:osz, ca // 2:cb // 2, 0],
                        scalar1=4, scalar2=0x6400,
                        op0=A.logical_shift_right, op1=A.bitwise_or)
                    # op1: all i16 lanes: (v & 15) | 0x6400
                    nc.vector.tensor_scalar(out=q16[:osz, ca:cb],
                                            in0=q16[:osz, ca:cb],
                                            scalar1=15, scalar2=0x6400,
                                            op0=A.bitwise_and, op1=A.bitwise_or)
                    # dequant in place: w64 = vf * s64 + zs64  (== 64(q-z)s)
                    for g in range(sl * gsl, (sl + 1) * gsl):
                        nc.vector.tensor_scalar(
                            out=vf16[:osz, g * GROUP:(g + 1) * GROUP],
                            in0=vf16[:osz, g * GROUP:(g + 1) * GROUP],
                            scalar1=s64[:osz, g:g + 1],
                            scalar2=zs64[:osz, g:g + 1],
                            op0=A.mult, op1=A.add)
                for sl in range(nslices):
                    ca, cb = sl * (K // nslices), (sl + 1) * (K // nslices)
                    jta, jtb = sl * (JT // nslices), (sl + 1) * (JT // nslices)
                    # fp8 hi on ACT, then its transpose on ACT HWDGE
                    nc.scalar.copy(out=whi8[:osz, ca:cb], in_=vf16[:osz, ca:cb])
                    nc.scalar.dma_start(out=wthi3[:, jta:jtb, o0:o0 + osz],
                                        in_=whi_u16[:osz, ca // 2:cb // 2],
                                        transpose=True)
                    # residual lo on DVE (mixed fp16-fp8 subtract, 1x), then
                    # its transpose on SP HWDGE
                    nc.vector.tensor_tensor(out=wlo8[:osz, ca:cb],
                                            in0=vf16[:osz, ca:cb],
                                            in1=whi8[:osz, ca:cb],
                                            op=A.subtract)
                    nc.sync.dma_start(out=wtlo3[:, jta:jtb, o0:o0 + osz],
                                      in_=wlo_u16[:osz, ca // 2:cb // 2],
                                      transpose=True)

            # ------- phase A: DR matmuls for up to 4 (ot, mtile) units --------
            pending_drains = []

            def emit_unit_mms(units):
                # one PSUM bank per unit (start=True zeroes the whole bank's
                # zero region, so independent accumulations never share)
                tiles = [pcol.tile([128, 512], f32, tag="pc", name="pc")
                         for _ in units]
                for st in range(3):
                    for jt in range(JT):
                        for (t, mi), ps in zip(units, tiles):
                            o0 = t * 128
                            osz = min(128, OC - o0)
                            bi = (mi // SUBS) % XRING
                            sub = mi % SUBS
                            xv = xhs[bi] if st != 1 else xls[bi]
                            wv = wthi_v if st != 2 else wtlo_v
                            nc.tensor.matmul(
                                ps[:, :osz],
                                lhsT=xv[:, jt, :, sub * 128:(sub + 1) * 128],
                                rhs=wv[:, jt, :, o0:o0 + osz],
                                start=(st == 0 and jt == 0),
                                stop=(st == 2 and jt == JT - 1),
                                perf_mode=DR)
                pending_drains.append((tiles, list(units)))

            stg_tiles = {}      # t -> [stage tile, pieces done]

            def flush_drains(keep=0):
                while len(pending_drains) > keep:
                    tiles, units = pending_drains.pop(0)
                    for (t, mi), ps in zip(units, tiles):
                        o0 = t * 128
                        osz = min(128, OC - o0)
                        if t not in stg_tiles:
                            stg_tiles[t] = [stgp.tile([128, AUNITS * 128], fp16,
                                                      tag="sa", name="sa"), 0]
                        sa = stg_tiles[t][0]
                        nc.scalar.mul(out=sa[:, mi * 128:mi * 128 + osz],
                                      in_=ps[:, :osz], mul=1.0 / WSCALE)
                        stg_tiles[t][1] += 1
                        if stg_tiles[t][1] == AUNITS:
                            sa_v = sa.rearrange("p (mi o) -> p mi o", mi=AUNITS)
                            nc.gpsimd.dma_start(
                                out=outA_v[:, :, o0:o0 + osz],
                                in_=sa_v[:, :, :osz])
                            del stg_tiles[t]

            # ------------- phase B: one full m-block -------------------------
            def emit_mblock(mb, load_xt=True):
                xh, xl = xhs[mb % XRING], xls[mb % XRING]
                if load_xt:
                    emit_xt(mb, 1)
                for sub in range(SUBS):
                    mi = mb * SUBS + sub
                    last = (mb == NB - 1 and sub == SUBS - 1)
                    chunks = [(0, 512), (512, 1024), (1024, OC)]
                    psts = [ppB.tile([128, 512], f32, tag=f"pp{j}",
                                     name=f"pp{j}") for j in range(2)]
                    psts.append(pcol.tile([128, 512], f32, tag="pc",
                                          name="pc"))
                    ob = obs[mi % 2]
                    if last:
                        # j-outer: early chunks drain while later ones matmul
                        for j, (c0, c1) in enumerate(chunks):
                            for st in range(3):
                                xv = xh if st != 1 else xl
                                wv = wthi_v if st != 2 else wtlo_v
                                for jt in range(JT):
                                    nc.tensor.matmul(
                                        psts[j][:, :c1 - c0],
                                        lhsT=xv[:, jt, :,
                                                sub * 128:(sub + 1) * 128],
                                        rhs=wv[:, jt, :, c0:c1],
                                        start=(st == 0 and jt == 0),
                                        stop=(st == 2 and jt == JT - 1),
                                        perf_mode=DR)
                            nc.scalar.mul(out=ob[:, c0:c1],
                                          in_=psts[j][:, :c1 - c0],
                                          mul=1.0 / WSCALE)
                            nc.scalar.dma_start(
                                out=out[mi * 128:(mi + 1) * 128, c0:c1],
                                in_=ob[:, c0:c1])
                    else:
                        for st in range(3):
                            xv = xh if st != 1 else xl
                            wv = wthi_v if st != 2 else wtlo_v
                            for jt in range(JT):
                                for j, (c0, c1) in enumerate(chunks):
                                    nc.tensor.matmul(
                                        psts[j][:, :c1 - c0],
                                        lhsT=xv[:, jt, :,
                                                sub * 128:(sub + 1) * 128],
                                        rhs=wv[:, jt, :, c0:c1],
                                        start=(st == 0 and jt == 0),
                                        stop=(st == 2 and jt == JT - 1),
                                        perf_mode=DR)
                        for j, (c0, c1) in enumerate(chunks):
                            nc.scalar.mul(out=ob[:, c0:c1],
                                          in_=psts[j][:, :c1 - c0],
                                          mul=1.0 / WSCALE)
                        nc.gpsimd.dma_start(out=out[mi * 128:(mi + 1) * 128, :],
                                            in_=ob)

            # ---------------- emission schedule ----------------
            def emit_xt(block, parts):
                xh, xl = xhs[block % XRING], xls[block % XRING]
                jp = JT // parts
                for part in range(parts):
                    ja, jb = part * jp, (part + 1) * jp
                    nc.gpsimd.dma_start(out=xh[:, ja:jb, :, :],
                                        in_=xhi_v5[:, block, ja:jb, :, :])
                    nc.gpsimd.dma_start(out=xl[:, ja:jb, :, :],
                                        in_=xlo_v5[:, block, ja:jb, :, :])

            # warm the ACT function table at t=0 so the auto-inserted
            # LoadActFuncSet doesn't sit in front of the first whi8 converts
            warm = deq.tile([128, 2], fp16, tag="warm", name="warm")
            nc.vector.memset(warm[:1, :2], 0.0)
            nc.scalar.copy(out=warm[:1, 1:2], in_=warm[:1, :1])

            # phase A: o-tiles with interleaved DR matmuls over m-blocks 0-4.
            SLICES = {0: 4, 1: 4, 2: 4, 3: 2}
            emit_loads(0, 4)
            emit_loads(1, 4)
            emit_xt(0, 4)
            emit_xt(1, 4)
            emit_deq(0, SLICES[0])
            emit_loads(2, 4)
            emit_deq(1, SLICES[1])
            emit_unit_mms([(0, 0), (0, 1)])
            emit_unit_mms([(0, 2), (0, 3)])
            flush_drains(keep=1)
            emit_loads(3, 2)
            emit_xt(2, 2)
            emit_deq(2, SLICES[2])
            emit_unit_mms([(1, 0), (1, 1)])
            flush_drains(keep=1)
            emit_unit_mms([(1, 2), (1, 3)])
            flush_drains(keep=1)
            emit_loads(4, 1)
            emit_xt(3, 2)
            emit_deq(3, SLICES[3])
            emit_unit_mms([(0, 4), (0, 5)])
            flush_drains(keep=1)
            emit_unit_mms([(1, 4), (1, 5)])
            flush_drains(keep=1)
            emit_unit_mms([(2, 0), (2, 1)])
            flush_drains(keep=1)
            emit_unit_mms([(2, 2), (2, 3)])
            flush_drains(keep=1)
            emit_unit_mms([(0, 6), (0, 7)])
            flush_drains(keep=1)
            emit_unit_mms([(1, 6), (1, 7)])
            flush_drains(keep=1)
            emit_unit_mms([(2, 4), (2, 5)])
            flush_drains(keep=1)
            emit_unit_mms([(2, 6), (2, 7)])
            flush_drains(keep=1)
            for t in range(4, OT):
                if t + 1 < OT:
                    emit_loads(t + 1, 1)
                emit_deq(t, 2)
                for mi0 in range(0, AUNITS, 2):
                    emit_unit_mms([(t - 1, mi0), (t - 1, mi0 + 1)])
                    flush_drains(keep=1)
            for mi0 in range(0, AUNITS, 2):
                emit_unit_mms([(OT - 1, mi0), (OT - 1, mi0 + 1)])
                flush_drains(keep=1)
            flush_drains(keep=0)

            # phase B: m-blocks 5..15
            for mb in range(XRING, NB):
                emit_mblock(mb)

    if not nc.is_finalized():
        nc.finalize()
    return nc


def kernel(x, qweight, scales, qzeros, group_size=128, **_unused):
    global LAST_RESULT
    import ml_dtypes
    from concourse.bass_utils import run_bass_kernel_spmd

    e4 = ml_dtypes.float8_e4m3

    if "nc" not in _CACHE:
        _CACHE["nc"] = _build_bass()
    nc = _CACHE["nc"]

    x2d = np.asarray(x).reshape(M, K)
    xT = np.ascontiguousarray(x2d.T).astype(np.float32)   # [K, M]
    x_hi = xT.astype(e4)
    x_lo = (xT - x_hi.astype(np.float32)).astype(e4)

    def pack(a):
        # [K, M] fp8 -> [128, nb, jt, i, m] block-contiguous,
        # k = 256*jt + 2*p + i, m = nb*XB + m'
        b = a.reshape(JT, 128, 2, NB, XB)          # jt, p, i, nb, xb
        return np.ascontiguousarray(
            b.transpose(1, 3, 0, 2, 4).reshape(128, -1)).view(np.uint8)

    xhi_p = pack(x_hi)
    xlo_p = pack(x_lo)
    qweight = np.asarray(qweight)
    scales = np.asarray(scales)
    qzeros = np.asarray(qzeros)

    in_maps = []
    for i in range(N_CORES):
        sl = slice(i * OC, (i + 1) * OC)
        in_maps.append({
            "xhi": xhi_p,
            "xlo": xlo_p,
            "qweight": np.ascontiguousarray(qweight[sl]),
            "scales": np.ascontiguousarray(scales[sl]),
            "qzeros": np.ascontiguousarray(qzeros[sl]),
        })

    res = run_bass_kernel_spmd(nc, in_maps, core_ids=list(range(N_CORES)),
                               **RUN_KWARGS)
    LAST_RESULT = res
    outs = [r["out"] for r in res.results]
    return np.concatenate(outs, axis=1).reshape(B, S, OUT_F).astype(np.float32)


# revision 61
# speedup vs baseline: 1.1249x; 1.0423x over previous
"""Trainium2 Bass kernel v4 for nn_CudaMixedBitLinear (GPTQ-style 4-bit linear).

out[b,s,o] = sum_k x[b,s,k] * W[o,k],  W[o,k] = (q[o,k] - z[o,g]) * s[o,g],
g = k // 128, q/z packed as nibbles (low nibble first) in int32 bytes.

Sharding: column-parallel over out_features across 8 cores (11008 -> 1376
per core), x replicated, outputs concatenated on host. No collectives.

v4 (from v3 at 589us, v2 fp16 at 639us): fp8 DoubleRow matmuls with the
phase-A pipeline rebalanced so the PE (the roofline engine at 460us busy)
never starves:
  - PE path: fp8e4m3 DoubleRow, 0.5 cycles/row, 256-k contraction per
    instruction.  Precision: 3 products per k-pair —
    (x_hi + x_lo) @ w_hi + x_hi @ w_lo — all sharing one PSUM
    accumulation at a global w-scale of 64 (rel err 1.3e-3 measured).
  - x is pre-split hi/lo fp8 on host, shipped block-contiguous
    [128, nb, jt, i, m] so each x-block load is 128 descriptors, not 4096
    (SWDGE descriptor generation runs on the Pool engine).
  - weight dequant per o-tile: DVE does unpack + 32 group scales +
    the w_lo subtract (fp16-fp8 mixed, 1x); the w_hi fp16->fp8 convert
    rides ACT (activation Copy); hi-transpose on ACT HWDGE, lo-transpose
    on SP HWDGE.  Transposes move uint16 PAIRS of adjacent-k fp8 values
    (1-byte dtypes can't XBAR-transpose; the pair layout is exactly the
    DoubleRow operand layout).
  - phase-A drains stage fp16 pieces into an SBUF tile per o-tile and
    leave as ONE Pool DMA per o-tile (11 DMAs instead of 88).
  - output tensor is fp16 (host upconverts to f32; adds ~3e-4 in
    quadrature), halving store traffic.
"""

import numpy as np

B, S, K = 2, 2048, 4096
OUT_F = 11008
N_CORES = 8
OC = OUT_F // N_CORES       # 1376 out features per core
GROUP = 128
GROUPS = K // GROUP         # 32
M = B * S                   # 4096 rows
JT = K // 256               # 16 pair-tiles (256 k's each)
OT = (OC + 127) // 128      # 11 o-tiles (last has 96 rows)
XB = 256                    # m columns per x block buffer
NB = M // XB                # 16 m-blocks
SUBS = XB // 128            # 2 m-tiles per block
WSCALE = 64.0               # w pre-scale so e4m3 stays in normal range
AUNITS = 8                  # phase A covers m-tiles 0..7 (m-blocks 0-3)
XRING = 4                   # x block ring depth

_CACHE = {}
RUN_KWARGS = {}   # test harness can inject e.g. dict(trace=True)
LAST_RESULT = None


def _build_bass():
    import concourse.bass as bass
    import concourse.bacc as bacc
    import concourse.mybir as mybir
    from concourse.tile import TileContext

    A = mybir.AluOpType
    fp16 = mybir.dt.float16
    f32 = mybir.dt.float32
    i32 = mybir.dt.int32
    i16 = mybir.dt.int16
    fp8 = mybir.dt.float8e4
    u16 = mybir.dt.uint16
    DR = mybir.MatmulPerfMode.DoubleRow

    nc = bacc.Bacc("TRN2", target_bir_lowering=False)
    xhi_d = nc.dram_tensor("xhi", [128, NB * JT * 2 * XB], fp8,
                           kind="ExternalInput")
    xlo_d = nc.dram_tensor("xlo", [128, NB * JT * 2 * XB], fp8,
                           kind="ExternalInput")
    qw = nc.dram_tensor("qweight", [OC, K // 2], i32, kind="ExternalInput")
    sc = nc.dram_tensor("scales", [OC, GROUPS], fp16, kind="ExternalInput")
    qz = nc.dram_tensor("qzeros", [OC, GROUPS // 2], i32, kind="ExternalInput")
    out = nc.dram_tensor("out", [M, OC], fp16, kind="ExternalOutput")

    # static SBUF: resident w^T hi/lo (uint16 pairs), x^T rings, out ring
    wthi = nc.alloc_sbuf_tensor("wthi", [128, JT * OC], u16).ap()
    wtlo = nc.alloc_sbuf_tensor("wtlo", [128, JT * OC], u16).ap()
    wthi3 = wthi.rearrange("p (jt oc) -> p jt oc", jt=JT)
    wtlo3 = wtlo.rearrange("p (jt oc) -> p jt oc", jt=JT)
    # fp8 views for the matmul rhs: [p, jt, i, o], i stride 1, o stride 2
    wthi_v = wthi.bitcast(fp8).rearrange("p (jt oc i) -> p jt i oc", jt=JT, i=2)
    wtlo_v = wtlo.bitcast(fp8).rearrange("p (jt oc i) -> p jt i oc", jt=JT, i=2)

    xhs = [nc.alloc_sbuf_tensor(f"xhbuf{i}", [128, JT, 2, XB], fp8).ap()
           for i in range(XRING)]
    xls = [nc.alloc_sbuf_tensor(f"xlbuf{i}", [128, JT, 2, XB], fp8).ap()
           for i in range(XRING)]
    obs = [nc.alloc_sbuf_tensor(f"obbuf{i}", [128, OC], fp16).ap()
           for i in range(2)]

    # x DRAM views: block-contiguous [p, nb, jt, i, m]
    xhi_v5 = xhi_d[:, :].rearrange("p (nb jt i m) -> p nb jt i m",
                                   nb=NB, jt=JT, i=2)
    xlo_v5 = xlo_d[:, :].rearrange("p (nb jt i m) -> p nb jt i m",
                                   nb=NB, jt=JT, i=2)
    # phase-A staged-output DRAM view: rows 0..AUNITS*128 as [p, mi, o]
    outA_v = out[0:AUNITS * 128, :].rearrange("(mi p) o -> p mi o", p=128)
    with TileContext(nc) as tc:
        with (
            tc.tile_pool(name="deq", bufs=3) as deq,
            tc.tile_pool(name="qp", bufs=3) as qp,
            tc.tile_pool(name="w8p", bufs=3) as w8p,
            tc.tile_pool(name="wlo", bufs=2) as wlop,
            tc.tile_pool(name="stgA", bufs=2) as stgp,
            tc.tile_pool(name="pCol", bufs=4, space="PSUM") as pcol,
            tc.tile_pool(name="psB", bufs=2, space="PSUM") as ppB,
        ):
            # ------------- phase A: load one o-tile ---------------------------
            loaded = {}

            def emit_loads(t, nslices=1):
                eng = nc.sync
                o0 = t * 128
                osz = min(128, OC - o0)
                s_t = deq.tile([128, GROUPS], fp16, tag="s", name="s_t")
                eng.dma_start(out=s_t[:osz], in_=sc[o0:o0 + osz, :])
                z_t = deq.tile([128, GROUPS // 2], i32, tag="z", name="z_t")
                eng.dma_start(out=z_t[:osz], in_=qz[o0:o0 + osz, :])
                q_t = qp.tile([128, K // 2], i32, tag="q", name="q_t")
                ksl = (K // 2) // nslices
                for sl in range(nslices):
                    eng.dma_start(out=q_t[:osz, sl * ksl:(sl + 1) * ksl],
                                  in_=qw[o0:o0 + osz, sl * ksl:(sl + 1) * ksl])
                loaded[t] = (s_t, z_t, q_t)

            # ------------- phase A: dequant one o-tile (k-sliced) -------------
            def emit_deq(t, nslices):
                o0 = t * 128
                osz = min(128, OC - o0)
                s_t, z_t, q_t = loaded.pop(t)

                # zeros -> fp16(1024+z) via mantissa-OR trick (i32 lanes)
                z1 = deq.tile([128, GROUPS // 2], i32, tag="z1", name="z1")
                z2 = deq.tile([128, GROUPS // 2], i32, tag="z2", name="z2")
                nc.vector.tensor_scalar(out=z1[:osz], in0=z_t[:osz], scalar1=15,
                                        scalar2=0x64006400, op0=A.bitwise_and,
                                        op1=A.bitwise_or)
                nc.vector.tensor_scalar(out=z2[:osz], in0=z_t[:osz], scalar1=12,
                                        scalar2=0x000F0000,
                                        op0=A.logical_shift_left, op1=A.bitwise_and)
                nc.vector.tensor_tensor(out=z1[:osz], in0=z1[:osz], in1=z2[:osz],
                                        op=A.bitwise_or)
                zf = z1.bitcast(fp16)   # [128, GROUPS] = 1024 + z

                # per-group fp32 scalars: s64 = 64*s ; zs64 = -(1024+z)*64*s
                s64 = deq.tile([128, GROUPS], f32, tag="s64", name="s64")
                nc.vector.tensor_scalar(out=s64[:osz], in0=s_t[:osz],
                                        scalar1=WSCALE, scalar2=None, op0=A.mult)
                zs64 = deq.tile([128, GROUPS], f32, tag="zs64", name="zs64")
                nc.vector.tensor_tensor(out=zs64[:osz], in0=zf[:osz],
                                        in1=s_t[:osz], op=A.mult)
                nc.vector.tensor_scalar(out=zs64[:osz], in0=zs64[:osz],
                                        scalar1=-WSCALE, scalar2=None, op0=A.mult)

                # junky unpack fully in place over the q tile; then per-group
                # dequant in place (fp16, scaled by 64).
                q16 = q_t.bitcast(i16)          # [128, K] lanes (b, 0, ...)
                q16_2 = q16.rearrange("p (c two) -> p c two", two=2)
                vf16 = q_t.bitcast(fp16)        # [128, K]
                whi8 = w8p.tile([128, K], fp8, tag="whi", name="whi8")
                wlo8 = wlop.tile([128, K], fp8, tag="wlo", name="wlo8")
                whi_u16 = whi8.bitcast(u16)     # [128, K//2]
                wlo_u16 = wlo8.bitcast(u16)
                gsl = GROUPS // nslices
                for sl in range(nslices):
                    ca, cb = sl * (K // nslices), (sl + 1) * (K // nslices)
                    # op2: odd lanes = (even_lane >> 4) | 0x6400
                    nc.vector.tensor_scalar(
                        out=q16_2[:osz, ca // 2:cb // 2, 1],
                        in0=q16_2[:osz, ca // 2:cb // 2, 0],
                        scalar1=4, scalar2=0x6400,
                        op0=A.logical_shift_right, op1=A.bitwise_or)
                    # op1: all i16 lanes: (v & 15) | 0x6400
                    nc.vector.tensor_scalar(out=q16[:osz, ca:cb],
                                            in0=q16[:osz, ca:cb],
                                            scalar1=15, scalar2=0x6400,
                                            op0=A.bitwise_and, op1=A.bitwise_or)
                    # dequant in place: w64 = vf * s64 + zs64  (== 64(q-z)s)
                    for g in range(sl * gsl, (sl + 1) * gsl):
                        nc.vector.tensor_scalar(
                            out=vf16[:osz, g * GROUP:(g + 1) * GROUP],
                            in0=vf16[:osz, g * GROUP:(g + 1) * GROUP],
                            scalar1=s64[:osz, g:g + 1],
                            scalar2=zs64[:osz, g:g + 1],
                            op0=A.mult, op1=A.add)
                for sl in range(nslices):
                    ca, cb = sl * (K // nslices), (sl + 1) * (K // nslices)
                    jta, jtb = sl * (JT // nslices), (sl + 1) * (JT // nslices)
                    # fp8 hi on ACT, then its transpose on ACT HWDGE
                    nc.scalar.copy(out=whi8[:osz, ca:cb], in_=vf16[:osz, ca:cb])
                    nc.scalar.dma_start(out=wthi3[:, jta:jtb, o0:o0 + osz],
                                        in_=whi_u16[:osz, ca // 2:cb // 2],
                                        transpose=True)
                    # residual lo on DVE (mixed fp16-fp8 subtract, 1x), then
                    # its transpose on SP HWDGE
                    nc.vector.tensor_tensor(out=wlo8[:osz, ca:cb],
                                            in0=vf16[:osz, ca:cb],
                                            in1=whi8[:osz, ca:cb],
                                            op=A.subtract)
                    nc.sync.dma_start(out=wtlo3[:, jta:jtb, o0:o0 + osz],
                                      in_=wlo_u16[:osz, ca // 2:cb // 2],
                                      transpose=True)

            # ------- phase A: DR matmuls for up to 4 (ot, mtile) units --------
            pending_drains = []

            def emit_unit_mms(units):
                # one PSUM bank per unit (start=True zeroes the whole bank's
                # zero region, so independent accumulations never share)
                tiles = [pcol.tile([128, 512], f32, tag="pc", name="pc")
                         for _ in units]
                for st in range(3):
                    for jt in range(JT):
                        for (t, mi), ps in zip(units, tiles):
                            o0 = t * 128
                            osz = min(128, OC - o0)
                            bi = (mi // SUBS) % XRING
                            sub = mi % SUBS
                            xv = xhs[bi] if st != 1 else xls[bi]
                            wv = wthi_v if st != 2 else wtlo_v
                            nc.tensor.matmul(
                                ps[:, :osz],
                                lhsT=xv[:, jt, :, sub * 128:(sub + 1) * 128],
                                rhs=wv[:, jt, :, o0:o0 + osz],
                                start=(st == 0 and jt == 0),
                                stop=(st == 2 and jt == JT - 1),
                                perf_mode=DR)
                pending_drains.append((tiles, list(units)))

            stg_tiles = {}      # t -> [stage tile, pieces done]

            def flush_drains(keep=0):
                while len(pending_drains) > keep:
                    tiles, units = pending_drains.pop(0)
                    for (t, mi), ps in zip(units, tiles):
                        o0 = t * 128
                        osz = min(128, OC - o0)
                        if t not in stg_tiles:
                            stg_tiles[t] = [stgp.tile([128, AUNITS * 128], fp16,
                                                      tag="sa", name="sa"), 0]
                        sa = stg_tiles[t][0]
                        nc.scalar.mul(out=sa[:, mi * 128:mi * 128 + osz],
                                      in_=ps[:, :osz], mul=1.0 / WSCALE)
                        stg_tiles[t][1] += 1
                        if stg_tiles[t][1] == AUNITS:
                            sa_v = sa.rearrange("p (mi o) -> p mi o", mi=AUNITS)
                            nc.gpsimd.dma_start(
                                out=outA_v[:, :, o0:o0 + osz],
                                in_=sa_v[:, :, :osz])
                            del stg_tiles[t]

            # ------------- phase B: one full m-block -------------------------
            def emit_mblock(mb, load_xt=True):
                xh, xl = xhs[mb % XRING], xls[mb % XRING]
                if load_xt:
                    emit_xt(mb, 1)
                for sub in range(SUBS):
                    mi = mb * SUBS + sub
                    last = (mb == NB - 1 and sub == SUBS - 1)
                    chunks = [(0, 512), (512, 1024), (1024, OC)]
                    psts = [ppB.tile([128, 512], f32, tag=f"pp{j}",
                                     name=f"pp{j}") for j in range(2)]
                    psts.append(pcol.tile([128, 512], f32, tag="pc",
                                          name="pc"))
                    ob = obs[mi % 2]
                    if last:
                        # j-outer: early chunks drain while later ones matmul
                        for j, (c0, c1) in enumerate(chunks):
                            for st in range(3):
                                xv = xh if st != 1 else xl
                                wv = wthi_v if st != 2 else wtlo_v
                                for jt in range(JT):
                                    nc.tensor.matmul(
                                        psts[j][:, :c1 - c0],
                                        lhsT=xv[:, jt, :,
                                                sub * 128:(sub + 1) * 128],
                                        rhs=wv[:, jt, :, c0:c1],
                                        start=(st == 0 and jt == 0),
                                        stop=(st == 2 and jt == JT - 1),
                                        perf_mode=DR)
                            nc.scalar.mul(out=ob[:, c0:c1],
                                          in_=psts[j][:, :c1 - c0],
                                          mul=1.0 / WSCALE)
                            nc.scalar.dma_start(
                                out=out[mi * 128:(mi + 1) * 128, c0:c1],
                                in_=ob[:, c0:c1])
                    else:
                        for st in range(3):
                            xv = xh if st != 1 else xl
                            wv = wthi_v if st != 2 else wtlo_v
                            for jt in range(JT):
                                for j, (c0, c1) in enumerate(chunks):
                                    nc.tensor.matmul(
                                        psts[j][:, :c1 - c0],
                                        lhsT=xv[:, jt, :,
                                                sub * 128:(sub + 1) * 128],
                                        rhs=wv[:, jt, :, c0:c1],
                                        start=(st == 0 and jt == 0),
                                        stop=(st == 2 and jt == JT - 1),
                                        perf_mode=DR)
                        for j, (c0, c1) in enumerate(chunks):
                            nc.scalar.mul(out=ob[:, c0:c1],
                                          in_=psts[j][:, :c1 - c0],
                                          mul=1.0 / WSCALE)
                        nc.gpsimd.dma_start(out=out[mi * 128:(mi + 1) * 128, :],
                                            in_=ob)

            # ---------------- emission schedule ----------------
            def emit_xt(block, parts):
                xh, xl = xhs[block % XRING], xls[block % XRING]
                jp = JT // parts
                for part in range(parts):
                    ja, jb = part * jp, (part + 1) * jp
                    nc.gpsimd.dma_start(out=xh[:, ja:jb, :, :],
                                        in_=xhi_v5[:, block, ja:jb, :, :])
                    nc.gpsimd.dma_start(out=xl[:, ja:jb, :, :],
                                        in_=xlo_v5[:, block, ja:jb, :, :])

            # warm the ACT function table at t=0 so the auto-inserted
            # LoadActFuncSet doesn't sit in front of the first whi8 converts
            warm = deq.tile([128, 2], fp16, tag="warm", name="warm")
            nc.vector.memset(warm[:1, :2], 0.0)
            nc.scalar.copy(out=warm[:1, 1:2], in_=warm[:1, :1])

            # phase A: o-tiles with interleaved DR matmuls over m-blocks 0-4.
            SLICES = {0: 4, 1: 4, 2: 4, 3: 2}
            emit_loads(0, 4)
            emit_loads(1, 4)
            emit_loads(2, 4)
            emit_xt(0, 4)
            emit_xt(1, 4)
            emit_deq(0, SLICES[0])
            emit_deq(1, SLICES[1])
            emit_unit_mms([(0, 0), (0, 1)])
            emit_unit_mms([(0, 2), (0, 3)])
            flush_drains(keep=1)
            emit_loads(3, 2)
            emit_xt(2, 2)
            emit_deq(2, SLICES[2])
            emit_unit_mms([(1, 0), (1, 1)])
            flush_drains(keep=1)
            emit_unit_mms([(1, 2), (1, 3)])
            flush_drains(keep=1)
            emit_loads(4, 1)
            emit_xt(3, 2)
            emit_deq(3, SLICES[3])
            emit_unit_mms([(0, 4), (0, 5)])
            flush_drains(keep=1)
            emit_unit_mms([(1, 4), (1, 5)])
            flush_drains(keep=1)
            emit_unit_mms([(0, 6), (0, 7)])
            flush_drains(keep=1)
            emit_unit_mms([(1, 6), (1, 7)])
            flush_drains(keep=1)
            emit_unit_mms([(2, 0), (2, 1)])
            flush_drains(keep=1)
            emit_unit_mms([(2, 2), (2, 3)])
            flush_drains(keep=1)
            emit_unit_mms([(2, 4), (2, 5)])
            flush_drains(keep=1)
            emit_unit_mms([(2, 6), (2, 7)])
            flush_drains(keep=1)
            for t in range(4, OT):
                if t + 1 < OT:
                    emit_loads(t + 1, 1)
                emit_deq(t, 4)
                for mi0 in range(0, AUNITS, 2):
                    emit_unit_mms([(t - 1, mi0), (t - 1, mi0 + 1)])
                    flush_drains(keep=1)
            for mi0 in range(0, AUNITS, 2):
                emit_unit_mms([(OT - 1, mi0), (OT - 1, mi0 + 1)])
                flush_drains(keep=1)
            flush_drains(keep=0)

            # phase B: m-blocks 5..15
            for mb in range(XRING, NB):
                emit_mblock(mb)

    if not nc.is_finalized():
        nc.finalize()
    return nc


def kernel(x, qweight, scales, qzeros, group_size=128, **_unused):
    global LAST_RESULT
    import ml_dtypes
    from concourse.bass_utils import run_bass_kernel_spmd

    e4 = ml_dtypes.float8_e4m3

    if "nc" not in _CACHE:
        _CACHE["nc"] = _build_bass()
    nc = _CACHE["nc"]

    x2d = np.asarray(x).reshape(M, K)
    xT = np.ascontiguousarray(x2d.T).astype(np.float32)   # [K, M]
    x_hi = xT.astype(e4)
    x_lo = (xT - x_hi.astype(np.float32)).astype(e4)

    def pack(a):
        # [K, M] fp8 -> [128, nb, jt, i, m] block-contiguous,
        # k = 256*jt + 2*p + i, m = nb*XB + m'
        b = a.reshape(JT, 128, 2, NB, XB)          # jt, p, i, nb, xb
        return np.ascontiguousarray(
            b.transpose(1, 3, 0, 2, 4).reshape(128, -1)).view(np.uint8)

    xhi_p = pack(x_hi)
    xlo_p = pack(x_lo)
    qweight = np.asarray(qweight)
    scales = np.asarray(scales)
    qzeros = np.asarray(qzeros)

    in_maps = []
    for i in range(N_CORES):
        sl = slice(i * OC, (i + 1) * OC)
        in_maps.append({
            "xhi": xhi_p,
            "xlo": xlo_p,
            "qweight": np.ascontiguousarray(qweight[sl]),
            "scales": np.ascontiguousarray(scales[sl]),
            "qzeros": np.ascontiguousarray(qzeros[sl]),
        })

    res = run_bass_kernel_spmd(nc, in_maps, core_ids=list(range(N_CORES)),
                               **RUN_KWARGS)
    LAST_RESULT = res
    outs = [r["out"] for r in res.results]
    return np.concatenate(outs, axis=1).reshape(B, S, OUT_F).astype(np.float32)
